# revision 14
# baseline (speedup 1.0000x reference)
"""Trainium2 Bass kernel for CustomRBF forward:

    out[i] = w * exp(-gamma * ||X[i] - centroid||^2) + b

Design (per core, data-parallel over 8 cores):
  - DMA X in natural layout [128 samples (partitions), 128 feats (free)],
    16 sample-tiles (1 MB) per dma_start.
  - TensorE transposes each [128,128] tile into PSUM -> [feat, sample] layout.
  - ScalarE fused subtract+square: activation(Square, bias=-c) PSUM->SBUF
    (bias is per-partition = per-feature in transposed layout).
  - TensorE reduce over features (partitions): matmul with the squared tile
    as stationary and a ones-vector moving -> out [128,1] = per-sample sums,
    one column per tile into a PSUM [128, 512] accumulator tile.
  - ScalarE Exp (scale=-gamma) PSUM->SBUF, VectorE tensor_scalar (*w + b).
  - TensorE transposes the [128, C] result back (chunks of 128 cols) so the
    output DMA is contiguous; VectorE copies PSUM->SBUF; DMA out.

Sharding: cores 0-6 take contiguous 125056-sample slices; core 7 takes the
last 125056 samples (overlapping core 6 by 448 samples so every core gets
exactly 977 full 128-sample tiles). The overlap is recomputed identically
and overwritten at gather time.
"""

import sys

sys.path.insert(0, "/opt/trn_rl_repo")

import numpy as np

D = 128          # feature dim
P = 128          # SBUF partitions
GAMMA = 1.0 / D
N_CORES = 8
TILES = 977      # 128-sample tiles per core
SHARD = TILES * P           # 125056
N_TOTAL = 1000000
GROUP = 16       # tiles per DMA
HGROUP = 8       # tiles per half-group (one PSUM tr tile)
PO_TILES = 256   # tiles per PSUM accumulator tile (2 cols each: [sum, 0])

_NC_CACHE = {}


def _build(tiles=TILES, po_tiles=PO_TILES):
    from contextlib import ExitStack

    import concourse.tile as tile
    from concourse import bacc, mybir

    f32 = mybir.dt.float32
    f32r = mybir.dt.float32r
    Act = mybir.ActivationFunctionType
    Alu = mybir.AluOpType

    n = tiles * P
    nc = bacc.Bacc("TRN2", target_bir_lowering=False, debug=False,
                   num_devices=N_CORES)
    xh = nc.declare_dram_parameter("x", [n, D], f32, isOutput=False)
    negch = nc.declare_dram_parameter("negc", [P, 1], f32, isOutput=False)
    identh = nc.declare_dram_parameter("ident", [P, D], f32, isOutput=False)
    onesh = nc.declare_dram_parameter("ones", [P, 2], f32, isOutput=False)
    wh = nc.declare_dram_parameter("wvec", [P, 1], f32, isOutput=False)
    bh = nc.declare_dram_parameter("bvec", [P, 1], f32, isOutput=False)
    outh = nc.declare_dram_parameter("out", [n], f32, isOutput=True)

    x_v = xh[:, :].rearrange("(t p) k -> p t k", p=P)  # [128, tiles, 128]

    with ExitStack() as ctx:
        tc = ctx.enter_context(tile.TileContext(nc))
        singles = ctx.enter_context(tc.tile_pool(name="singles", bufs=1))
        xin = ctx.enter_context(tc.tile_pool(name="xin", bufs=4))
        yp = ctx.enter_context(tc.tile_pool(name="y", bufs=3))
        resp = ctx.enter_context(tc.tile_pool(name="res", bufs=2))
        rtp = ctx.enter_context(tc.tile_pool(name="rt", bufs=3))
        trp = ctx.enter_context(tc.tile_pool(name="tr", bufs=2, space="PSUM"))
        pop = ctx.enter_context(tc.tile_pool(name="po", bufs=2, space="PSUM"))
        ttp = ctx.enter_context(tc.tile_pool(name="tt", bufs=2, space="PSUM"))

        negc_s = singles.tile([P, 1], f32)
        nc.sync.dma_start(out=negc_s, in_=negch[:, :])
        ident_s = singles.tile([P, D], f32)
        nc.sync.dma_start(out=ident_s, in_=identh[:, :])
        ones_s = singles.tile([P, 2], f32)
        nc.sync.dma_start(out=ones_s, in_=onesh[:, :])
        ones_r = singles.tile([P, 2], f32r)
        nc.vector.tensor_copy(out=ones_r, in_=ones_s)
        wv_s = singles.tile([P, 1], f32)
        nc.sync.dma_start(out=wv_s, in_=wh[:, :])
        bv_s = singles.tile([P, 1], f32)
        nc.sync.dma_start(out=bv_s, in_=bh[:, :])

        state = {"po": None, "ntiles": 0, "base": 0}

        def finalize_po():
            po, T, base = state["po"], state["ntiles"], state["base"]
            C = 2 * T  # live columns (every odd column is a discarded zero)
            res = resp.tile([P, 2 * po_tiles], f32, name="res", tag="res")
            nc.scalar.activation(out=res[:, :C], in_=po[:, :C],
                                 func=Act.Exp, scale=-GAMMA, bias=0.0)
            nc.vector.tensor_scalar(out=res[:, :C], in0=res[:, :C],
                                    scalar1=wv_s[:, :], scalar2=bv_s[:, :],
                                    op0=Alu.mult, op1=Alu.add)
            c0 = 0
            while c0 < C:
                ncol = min(P, C - c0)
                nt = ncol // 2  # tiles in this chunk
                tt = ttp.tile([P, D], f32, name="tt", tag="tt")
                nc.tensor.transpose(out=tt[:ncol, :], in_=res[:, c0:c0 + ncol],
                                    identity=ident_s[:, :])
                rt = rtp.tile([P, D], f32, name="rt", tag="rt")
                nc.vector.tensor_copy(out=rt[:ncol, :], in_=tt[:ncol, :])
                # even partitions of rt hold real tiles; odd hold zeros
                src = rt.rearrange("(t two) f -> t two f", two=2)[:nt, 0, :]
                dest = outh[base + (c0 // 2) * P:
                            base + (c0 // 2 + nt) * P].rearrange(
                    "(t p) -> t p", p=P)
                nc.sync.dma_start(out=dest, in_=src)
                c0 += ncol
            state["po"] = None
            state["ntiles"] = 0

        t_done = 0
        while t_done < tiles:
            gt = min(GROUP, tiles - t_done)
            xt = xin.tile([P, GROUP, D], f32, name="xt", tag="xt")
            nc.sync.dma_start(out=xt[:, :gt, :],
                              in_=x_v[:, t_done:t_done + gt, :])
            hg = 0
            while hg < gt:
                ht = min(HGROUP, gt - hg)
                tr = trp.tile([P, HGROUP * D], f32, name="tr", tag="tr")
                for j in range(ht):
                    nc.tensor.transpose(out=tr[:, j * D:(j + 1) * D],
                                        in_=xt[:, hg + j, :],
                                        identity=ident_s[:, :])
                y = yp.tile([P, HGROUP * D], f32r, name="y", tag="y")
                nc.scalar.activation(out=y[:, :ht * D], in_=tr[:, :ht * D],
                                     func=Act.Square, bias=negc_s[:, :],
                                     scale=1.0)
                for j in range(ht):
                    if state["po"] is None:
                        state["po"] = pop.tile([P, 2 * po_tiles], f32,
                                               name="po", tag="po")
                        state["ntiles"] = 0
                        state["base"] = (t_done + hg + j) * P
                    col = 2 * state["ntiles"]
                    nc.tensor.matmul(
                        out=state["po"][:, col:col + 2],
                        lhsT=y[:, j * D:(j + 1) * D],
                        rhs=ones_r[:, :],
                        start=True, stop=True)
                    state["ntiles"] += 1
                    if state["ntiles"] == po_tiles:
                        finalize_po()
                hg += ht
            t_done += gt
        if state["po"] is not None:
            finalize_po()

    nc.finalize()
    return nc


def _get_nc(tiles=TILES):
    if tiles not in _NC_CACHE:
        _NC_CACHE[tiles] = _build(tiles)
    return _NC_CACHE[tiles]


def _make_const_inputs(centroid, w, b):
    centroid = np.asarray(centroid, dtype=np.float32).reshape(D)
    w = np.asarray(w, dtype=np.float32).reshape(-1)[0]
    b = np.asarray(b, dtype=np.float32).reshape(-1)[0]
    return {
        "negc": (-centroid).reshape(P, 1).copy(),
        "ident": np.eye(P, dtype=np.float32),
        "ones": np.tile(np.array([1.0, 0.0], dtype=np.float32), (P, 1)),
        "wvec": np.full((P, 1), w, dtype=np.float32),
        "bvec": np.full((P, 1), b, dtype=np.float32),
    }


def kernel(X, centroid, w, b, _trace=False, _trace_kwargs=None):
    from concourse.bass_utils import run_bass_kernel_spmd

    X = np.asarray(X)
    assert X.shape == (N_TOTAL, D), X.shape
    if X.dtype != np.float32:
        X = X.astype(np.float32)

    consts = _make_const_inputs(centroid, w, b)
    starts = [i * SHARD for i in range(N_CORES - 1)] + [N_TOTAL - SHARD]
    in_maps = [dict(consts, x=X[s:s + SHARD]) for s in starts]

    nc = _get_nc()
    kw = {}
    if _trace:
        kw = dict(trace=True, **(_trace_kwargs or {}))
    res = run_bass_kernel_spmd(nc, in_maps, list(range(N_CORES)), **kw)

    out = np.empty(N_TOTAL, dtype=np.float32)
    for i, s in enumerate(starts):
        out[s:s + SHARD] = res.results[i]["out"]
    if _trace:
        return out, res
    return out


# revision 17
# speedup vs baseline: 239.1061x; 239.1061x over previous
"""Trainium2 Bass kernel for CustomRBF forward:

    out[i] = w * exp(-gamma * ||X[i] - centroid||^2) + b

Design (per core, data-parallel over 8 cores):
  - DMA X in natural layout [128 samples (partitions), 128 feats (free)],
    16 sample-tiles (1 MB) per dma_start.
  - TensorE transposes each [128,128] tile into PSUM -> [feat, sample] layout.
  - ScalarE fused subtract+square: activation(Square, bias=-c) PSUM->SBUF
    (bias is per-partition = per-feature in transposed layout).
  - TensorE reduce over features (partitions): fp32r matmul with the squared
    tile as stationary and a [1,0] two-column moving operand -> out [128,2]
    (per-sample sums + a zero column), two columns per tile into a PSUM
    [128, 512] accumulator (fp32r needs even N and 8B-aligned dst).
  - ScalarE Exp (scale=-gamma) PSUM->SBUF, VectorE tensor_scalar (*w + b).
  - TensorE transposes the result back (chunks of 128 cols) so the output
    DMA is contiguous; VectorE copies PSUM->SBUF; DMA out (even partitions).

Sharding: cores 0-6 take contiguous 125056-sample slices; core 7 takes the
last 125056 samples (overlapping core 6 by 448 samples so every core gets
exactly 977 full 128-sample tiles). The overlap is recomputed identically
and overwritten at gather time.

`repeats` re-emits the whole pipeline R times in one NEFF (same data, same
output) — used only for differential wall-clock timing of the steady state.
"""

import sys

sys.path.insert(0, "/opt/trn_rl_repo")

import numpy as np

D = 128          # feature dim
P = 128          # SBUF partitions
GAMMA = 1.0 / D
N_CORES = 8
TILES = 977      # 128-sample tiles per core
SHARD = TILES * P           # 125056
N_TOTAL = 1000000
GROUP = 16       # tiles per DMA
HGROUP = 8       # tiles per half-group (one PSUM tr tile)
PO_TILES = 256   # tiles per PSUM accumulator tile (2 cols each: [sum, 0])

_NC_CACHE = {}


def _build(tiles=TILES, po_tiles=PO_TILES, repeats=1):
    from contextlib import ExitStack

    import concourse.tile as tile
    from concourse import bacc, mybir

    f32 = mybir.dt.float32
    f32r = mybir.dt.float32r
    Act = mybir.ActivationFunctionType
    Alu = mybir.AluOpType

    n = tiles * P
    nc = bacc.Bacc("TRN2", target_bir_lowering=False, debug=False,
                   num_devices=N_CORES)
    xh = nc.declare_dram_parameter("x", [n, D], f32, isOutput=False)
    negch = nc.declare_dram_parameter("negc", [P, 1], f32, isOutput=False)
    identh = nc.declare_dram_parameter("ident", [P, D], f32, isOutput=False)
    onesh = nc.declare_dram_parameter("ones", [P, 2], f32, isOutput=False)
    wh = nc.declare_dram_parameter("wvec", [P, 1], f32, isOutput=False)
    bh = nc.declare_dram_parameter("bvec", [P, 1], f32, isOutput=False)
    outh = nc.declare_dram_parameter("out", [n], f32, isOutput=True)

    x_v = xh[:, :].rearrange("(t p) k -> p t k", p=P)  # [128, tiles, 128]

    with ExitStack() as ctx:
        tc = ctx.enter_context(tile.TileContext(nc))
        singles = ctx.enter_context(tc.tile_pool(name="singles", bufs=1))
        xin = ctx.enter_context(tc.tile_pool(name="xin", bufs=4))
        yp = ctx.enter_context(tc.tile_pool(name="y", bufs=3))
        resp = ctx.enter_context(tc.tile_pool(name="res", bufs=2))
        rtp = ctx.enter_context(tc.tile_pool(name="rt", bufs=3))
        trp = ctx.enter_context(tc.tile_pool(name="tr", bufs=2, space="PSUM"))
        pop = ctx.enter_context(tc.tile_pool(name="po", bufs=2, space="PSUM"))
        ttp = ctx.enter_context(tc.tile_pool(name="tt", bufs=2, space="PSUM"))

        negc_s = singles.tile([P, 1], f32)
        nc.sync.dma_start(out=negc_s, in_=negch[:, :])
        ident_s = singles.tile([P, D], f32)
        nc.sync.dma_start(out=ident_s, in_=identh[:, :])
        ones_s = singles.tile([P, 2], f32)
        nc.sync.dma_start(out=ones_s, in_=onesh[:, :])
        ones_r = singles.tile([P, 2], f32r)
        nc.vector.tensor_copy(out=ones_r, in_=ones_s)
        wv_s = singles.tile([P, 1], f32)
        nc.sync.dma_start(out=wv_s, in_=wh[:, :])
        bv_s = singles.tile([P, 1], f32)
        nc.sync.dma_start(out=bv_s, in_=bh[:, :])

        state = {"po": None, "ntiles": 0, "base": 0}

        def finalize_po():
            po, T, base = state["po"], state["ntiles"], state["base"]
            C = 2 * T  # live columns (every odd column is a discarded zero)
            res = resp.tile([P, 2 * po_tiles], f32, name="res", tag="res")
            nc.scalar.activation(out=res[:, :C], in_=po[:, :C],
                                 func=Act.Exp, scale=-GAMMA, bias=0.0)
            nc.vector.tensor_scalar(out=res[:, :C], in0=res[:, :C],
                                    scalar1=wv_s[:, :], scalar2=bv_s[:, :],
                                    op0=Alu.mult, op1=Alu.add)
            c0 = 0
            while c0 < C:
                ncol = min(P, C - c0)
                nt = ncol // 2  # tiles in this chunk
                tt = ttp.tile([P, D], f32, name="tt", tag="tt")
                nc.tensor.transpose(out=tt[:ncol, :],
                                    in_=res[:, c0:c0 + ncol],
                                    identity=ident_s[:, :])
                rt = rtp.tile([P, D], f32, name="rt", tag="rt")
                nc.vector.tensor_copy(out=rt[:ncol, :], in_=tt[:ncol, :])
                # even partitions of rt hold real tiles; odd hold zeros
                src = rt.rearrange("(t two) f -> t two f", two=2)[:nt, 0, :]
                dest = outh[base + (c0 // 2) * P:
                            base + (c0 // 2 + nt) * P].rearrange(
                    "(t p) -> t p", p=P)
                nc.sync.dma_start(out=dest, in_=src)
                c0 += ncol
            state["po"] = None
            state["ntiles"] = 0

        for _rep in range(repeats):
            t_done = 0
            while t_done < tiles:
                gt = min(GROUP, tiles - t_done)
                xt = xin.tile([P, GROUP, D], f32, name="xt", tag="xt")
                nc.sync.dma_start(out=xt[:, :gt, :],
                                  in_=x_v[:, t_done:t_done + gt, :])
                hg = 0
                while hg < gt:
                    ht = min(HGROUP, gt - hg)
                    tr = trp.tile([P, HGROUP * D], f32, name="tr", tag="tr")
                    for j in range(ht):
                        nc.tensor.transpose(out=tr[:, j * D:(j + 1) * D],
                                            in_=xt[:, hg + j, :],
                                            identity=ident_s[:, :])
                    y = yp.tile([P, HGROUP * D], f32r, name="y", tag="y")
                    nc.scalar.activation(out=y[:, :ht * D], in_=tr[:, :ht * D],
                                         func=Act.Square, bias=negc_s[:, :],
                                         scale=1.0)
                    for j in range(ht):
                        if state["po"] is None:
                            state["po"] = pop.tile([P, 2 * po_tiles], f32,
                                                   name="po", tag="po")
                            state["ntiles"] = 0
                            state["base"] = (t_done + hg + j) * P
                        col = 2 * state["ntiles"]
                        nc.tensor.matmul(
                            out=state["po"][:, col:col + 2],
                            lhsT=y[:, j * D:(j + 1) * D],
                            rhs=ones_r[:, :],
                            start=True, stop=True)
                        state["ntiles"] += 1
                        if state["ntiles"] == po_tiles:
                            finalize_po()
                    hg += ht
                t_done += gt
            if state["po"] is not None:
                finalize_po()

    nc.finalize()
    return nc


def _get_nc(tiles=TILES):
    if tiles not in _NC_CACHE:
        _NC_CACHE[tiles] = _build(tiles)
    return _NC_CACHE[tiles]


def _make_const_inputs(centroid, w, b):
    centroid = np.asarray(centroid, dtype=np.float32).reshape(D)
    w = np.asarray(w, dtype=np.float32).reshape(-1)[0]
    b = np.asarray(b, dtype=np.float32).reshape(-1)[0]
    return {
        "negc": (-centroid).reshape(P, 1).copy(),
        "ident": np.eye(P, dtype=np.float32),
        "ones": np.tile(np.array([1.0, 0.0], dtype=np.float32), (P, 1)),
        "wvec": np.full((P, 1), w, dtype=np.float32),
        "bvec": np.full((P, 1), b, dtype=np.float32),
    }


def kernel(X, centroid, w, b, _trace=False, _trace_kwargs=None):
    from concourse.bass_utils import run_bass_kernel_spmd

    X = np.asarray(X)
    assert X.shape == (N_TOTAL, D), X.shape
    if X.dtype != np.float32:
        X = X.astype(np.float32)

    consts = _make_const_inputs(centroid, w, b)
    starts = [i * SHARD for i in range(N_CORES - 1)] + [N_TOTAL - SHARD]
    in_maps = [dict(consts, x=X[s:s + SHARD]) for s in starts]

    nc = _get_nc()
    kw = {}
    if _trace:
        kw = dict(trace=True, **(_trace_kwargs or {}))
    res = run_bass_kernel_spmd(nc, in_maps, list(range(N_CORES)), **kw)

    out = np.empty(N_TOTAL, dtype=np.float32)
    for i, s in enumerate(starts):
        out[s:s + SHARD] = res.results[i]["out"]
    if _trace:
        return out, res
    return out


# revision 20
# speedup vs baseline: 256.6566x; 1.0734x over previous
"""Trainium2 Bass kernel for CustomRBF forward:

    out[i] = w * exp(-gamma * ||X[i] - centroid||^2) + b

Design (per core, data-parallel over 8 cores):
  - DMA X in natural layout [128 samples (partitions), 128 feats (free)],
    16 sample-tiles (1 MB) per dma_start.
  - TensorE transposes each [128,128] tile into PSUM -> [feat, sample] layout.
  - ScalarE fused subtract+square: activation(Square, bias=-c) PSUM->SBUF
    (bias is per-partition = per-feature in transposed layout).
  - TensorE reduce over features (partitions): fp32r matmul with the squared
    tile as stationary and a [1,0] two-column moving operand -> out [128,2]
    (per-sample sums + a zero column), two columns per tile into a PSUM
    [128, 512] accumulator (fp32r needs even N and 8B-aligned dst).
  - ScalarE Exp (scale=-gamma) PSUM->SBUF, VectorE tensor_scalar (*w + b).
  - TensorE transposes the result back (chunks of 128 cols) so the output
    DMA is contiguous; VectorE copies PSUM->SBUF; DMA out (even partitions).

Sharding: cores 0-6 take contiguous 125056-sample slices; core 7 takes the
last 125056 samples (overlapping core 6 by 448 samples so every core gets
exactly 977 full 128-sample tiles). The overlap is recomputed identically
and overwritten at gather time.

`repeats` re-emits the whole pipeline R times in one NEFF (same data, same
output) — used only for differential wall-clock timing of the steady state.
"""

import sys

sys.path.insert(0, "/opt/trn_rl_repo")

import numpy as np

D = 128          # feature dim
P = 128          # SBUF partitions
GAMMA = 1.0 / D
N_CORES = 8
TILES = 977      # 128-sample tiles per core
SHARD = TILES * P           # 125056
N_TOTAL = 1000000
GROUP = 16       # tiles per DMA
HGROUP = 8       # tiles per half-group (one PSUM tr tile)
PO_TILES = 256   # tiles per PSUM accumulator tile (2 cols each: [sum, 0])

_NC_CACHE = {}


def _build(tiles=TILES, po_tiles=PO_TILES, repeats=1, group=GROUP,
           xin_bufs=4, y_bufs=3, tr_bufs=2):
    from contextlib import ExitStack

    import concourse.tile as tile
    from concourse import bacc, mybir

    f32 = mybir.dt.float32
    f32r = mybir.dt.float32r
    Act = mybir.ActivationFunctionType
    Alu = mybir.AluOpType

    n = tiles * P
    nc = bacc.Bacc("TRN2", target_bir_lowering=False, debug=False,
                   num_devices=N_CORES)
    xh = nc.declare_dram_parameter("x", [n, D], f32, isOutput=False)
    negch = nc.declare_dram_parameter("negc", [P, 1], f32, isOutput=False)
    identh = nc.declare_dram_parameter("ident", [P, D], f32, isOutput=False)
    onesh = nc.declare_dram_parameter("ones", [P, 2], f32, isOutput=False)
    wh = nc.declare_dram_parameter("wvec", [P, 1], f32, isOutput=False)
    bh = nc.declare_dram_parameter("bvec", [P, 1], f32, isOutput=False)
    outh = nc.declare_dram_parameter("out", [n], f32, isOutput=True)

    x_v = xh[:, :].rearrange("(t p) k -> p t k", p=P)  # [128, tiles, 128]

    with ExitStack() as ctx:
        tc = ctx.enter_context(tile.TileContext(nc))
        singles = ctx.enter_context(tc.tile_pool(name="singles", bufs=1))
        xin = ctx.enter_context(tc.tile_pool(name="xin", bufs=xin_bufs))
        yp = ctx.enter_context(tc.tile_pool(name="y", bufs=y_bufs))
        resp = ctx.enter_context(tc.tile_pool(name="res", bufs=2))
        rtp = ctx.enter_context(tc.tile_pool(name="rt", bufs=3))
        trp = ctx.enter_context(tc.tile_pool(name="tr", bufs=tr_bufs,
                                             space="PSUM"))
        pop = ctx.enter_context(tc.tile_pool(name="po", bufs=2, space="PSUM"))
        ttp = ctx.enter_context(tc.tile_pool(name="tt", bufs=2, space="PSUM"))

        negc_s = singles.tile([P, 1], f32)
        nc.sync.dma_start(out=negc_s, in_=negch[:, :])
        ident_s = singles.tile([P, D], f32)
        nc.sync.dma_start(out=ident_s, in_=identh[:, :])
        ones_s = singles.tile([P, 2], f32)
        nc.sync.dma_start(out=ones_s, in_=onesh[:, :])
        ones_r = singles.tile([P, 2], f32r)
        nc.vector.tensor_copy(out=ones_r, in_=ones_s)
        wv_s = singles.tile([P, 1], f32)
        nc.sync.dma_start(out=wv_s, in_=wh[:, :])
        bv_s = singles.tile([P, 1], f32)
        nc.sync.dma_start(out=bv_s, in_=bh[:, :])

        state = {"po": None, "ntiles": 0, "base": 0}

        def finalize_po():
            po, T, base = state["po"], state["ntiles"], state["base"]
            C = 2 * T  # live columns (every odd column is a discarded zero)
            res = resp.tile([P, 2 * po_tiles], f32, name="res", tag="res")
            nc.scalar.activation(out=res[:, :C], in_=po[:, :C],
                                 func=Act.Exp, scale=-GAMMA, bias=0.0)
            nc.vector.tensor_scalar(out=res[:, :C], in0=res[:, :C],
                                    scalar1=wv_s[:, :], scalar2=bv_s[:, :],
                                    op0=Alu.mult, op1=Alu.add)
            c0 = 0
            while c0 < C:
                ncol = min(P, C - c0)
                nt = ncol // 2  # tiles in this chunk
                tt = ttp.tile([P, D], f32, name="tt", tag="tt")
                nc.tensor.transpose(out=tt[:ncol, :],
                                    in_=res[:, c0:c0 + ncol],
                                    identity=ident_s[:, :])
                rt = rtp.tile([P, D], f32, name="rt", tag="rt")
                nc.vector.tensor_copy(out=rt[:ncol, :], in_=tt[:ncol, :])
                # even partitions of rt hold real tiles; odd hold zeros
                src = rt.rearrange("(t two) f -> t two f", two=2)[:nt, 0, :]
                dest = outh[base + (c0 // 2) * P:
                            base + (c0 // 2 + nt) * P].rearrange(
                    "(t p) -> t p", p=P)
                nc.sync.dma_start(out=dest, in_=src)
                c0 += ncol
            state["po"] = None
            state["ntiles"] = 0

        for _rep in range(repeats):
            t_done = 0
            while t_done < tiles:
                gt = min(group, tiles - t_done)
                xt = xin.tile([P, group, D], f32, name="xt", tag="xt")
                nc.sync.dma_start(out=xt[:, :gt, :],
                                  in_=x_v[:, t_done:t_done + gt, :])
                hg = 0
                while hg < gt:
                    ht = min(HGROUP, gt - hg)
                    tr = trp.tile([P, HGROUP * D], f32, name="tr", tag="tr")
                    for j in range(ht):
                        nc.tensor.transpose(out=tr[:, j * D:(j + 1) * D],
                                            in_=xt[:, hg + j, :],
                                            identity=ident_s[:, :])
                    y = yp.tile([P, HGROUP * D], f32r, name="y", tag="y")
                    nc.scalar.activation(out=y[:, :ht * D], in_=tr[:, :ht * D],
                                         func=Act.Square, bias=negc_s[:, :],
                                         scale=1.0)
                    for j in range(ht):
                        if state["po"] is None:
                            state["po"] = pop.tile([P, 2 * po_tiles], f32,
                                                   name="po", tag="po")
                            state["ntiles"] = 0
                            state["base"] = (t_done + hg + j) * P
                        col = 2 * state["ntiles"]
                        nc.tensor.matmul(
                            out=state["po"][:, col:col + 2],
                            lhsT=y[:, j * D:(j + 1) * D],
                            rhs=ones_r[:, :],
                            start=True, stop=True)
                        state["ntiles"] += 1
                        if state["ntiles"] == po_tiles:
                            finalize_po()
                    hg += ht
                t_done += gt
            if state["po"] is not None:
                finalize_po()

    nc.finalize()
    return nc


def _get_nc(tiles=TILES):
    if tiles not in _NC_CACHE:
        _NC_CACHE[tiles] = _build(tiles)
    return _NC_CACHE[tiles]


def _make_const_inputs(centroid, w, b):
    centroid = np.asarray(centroid, dtype=np.float32).reshape(D)
    w = np.asarray(w, dtype=np.float32).reshape(-1)[0]
    b = np.asarray(b, dtype=np.float32).reshape(-1)[0]
    return {
        "negc": (-centroid).reshape(P, 1).copy(),
        "ident": np.eye(P, dtype=np.float32),
        "ones": np.tile(np.array([1.0, 0.0], dtype=np.float32), (P, 1)),
        "wvec": np.full((P, 1), w, dtype=np.float32),
        "bvec": np.full((P, 1), b, dtype=np.float32),
    }


def kernel(X, centroid, w, b, _trace=False, _trace_kwargs=None):
    from concourse.bass_utils import run_bass_kernel_spmd

    X = np.asarray(X)
    assert X.shape == (N_TOTAL, D), X.shape
    if X.dtype != np.float32:
        X = X.astype(np.float32)

    consts = _make_const_inputs(centroid, w, b)
    starts = [i * SHARD for i in range(N_CORES - 1)] + [N_TOTAL - SHARD]
    in_maps = [dict(consts, x=X[s:s + SHARD]) for s in starts]

    nc = _get_nc()
    kw = {}
    if _trace:
        kw = dict(trace=True, **(_trace_kwargs or {}))
    res = run_bass_kernel_spmd(nc, in_maps, list(range(N_CORES)), **kw)

    out = np.empty(N_TOTAL, dtype=np.float32)
    for i, s in enumerate(starts):
        out[s:s + SHARD] = res.results[i]["out"]
    if _trace:
        return out, res
    return out
